# revision 12
# baseline (speedup 1.0000x reference)
"""Trainium2 Bass kernel for nn_Network_85220741087986 (v3: fp8 DoubleRow).

3-layer MLP: per layer  X[N,1024] @ W[1024,2048] -> per-group bilinear
interpolation on a 3x3 grid (ARITY=2) -> X[N,1024].

Reformulation (host-validated, rel err ~1.8e-3 vs fp32 reference):
  - Matmuls in fp8 e4m3 with perf_mode=DoubleRow (2 k-tiles per
    instruction); weights scaled by s=2^12 per layer, PSUM holds s*preact.
  - Activation in the {1, t, a} basis, t = clip(y,-1,1), a = |t|:
      f = (t0+lam)*ROW1 + (a0+mu)*ROW2 + C
      ROW_r = K_r0 + K_r1 t1 + K_r2 a1
    lam/mu solve a per-group 2x2 system (clamped to |.|<=8); clamped
    groups get a residual  + e1*t1 + e2*a1  and are permuted into the
    last group-tile of each layer so only that tile pays for it.  C and
    the per-feature bias fold into the next layer's bias shift; rows are
    scaled by sigma=1/16 so all fp16 intermediates stay in range, and the
    1/(s*sigma) descale rides the fp8-convert's free ACT scale slot.
  - Engine split per group-tile [128 groups x 2048 samples], chosen from
    measured op costs (DVE TS 4x ~660ns, TT 2x ~1150ns, ACT ~2000ns,
    scalar_tensor_tensor and PSUM-source DVE ops are 1x -> avoided):
      PE : 32 DoubleRow matmuls
      ACT: y1/y0 = psum + s*b (PSUM evac, fp16), h1/h2 row affines,
           fp8 convert (paired across 2 group-tiles, FD=4096)
      DVE: 2 symmetric clips (float bounds), scaled-abs ops (abs_max
           trick), 5 tensor_tensor ops, all 4x/2x SBUF fp16
  - 5-stage software pipeline across group-tiles (M, evac, clip/abs,
    row-affine, combine, convert) so no engine head-blocks on a
    same-tile cross-engine product.

Sharding: pure data parallel over 8 cores (2048 samples each), weights
and constants replicated.
"""

import os
import sys

import numpy as np
import ml_dtypes

for _p in ("/opt/trn_rl_repo", "/root/.axon_site/_ro/trn_rl_repo"):
    if os.path.isdir(_p) and _p not in sys.path:
        sys.path.append(_p)

import concourse.bacc as bacc
import concourse.mybir as mybir
import concourse.tile as tile
from concourse.alu_op_type import AluOpType
from concourse.bass_utils import run_bass_kernel_spmd


def _ensure_axon_hooks():
    """This image's antenv lacks axon_hooks; provide it (and register the
    NTFF profile hook) so trace=True doesn't crash run_bass_kernel_spmd."""
    import types

    try:
        import antenv.axon_hooks  # noqa: F401
        return
    except ImportError:
        pass
    mod = types.ModuleType("antenv.axon_hooks")
    _hook = [None]
    mod.get_axon_ntff_profile_hook = lambda: _hook[0]
    mod.set_axon_ntff_profile_hook = lambda h: _hook.__setitem__(0, h)
    sys.modules["antenv.axon_hooks"] = mod
    try:
        import antenv
        antenv.axon_hooks = mod
    except ImportError:
        pass
    try:
        from trn_agent_boot.trn_boot import _ntff_profile_via_ctypes
        so = "/opt/axon/libaxon_pjrt.so"
        if os.path.exists(so):
            _hook[0] = _ntff_profile_via_ctypes(so)
    except Exception:
        pass


_ensure_axon_hooks()

N_TOTAL = 16384
D_IN = 1024
F_OUT = 2048
N_LAYERS = 3
N_CORES = 8
NS = N_TOTAL // N_CORES   # samples per core
KT = D_IN // 128          # 8 contraction tiles
FT = F_OUT // 128         # 16 matmul-output feature tiles
GT = D_IN // 128          # 8 group tiles
PS = 512                  # psum chunk (one fp32 bank)
CL = 8.0                  # lambda/mu clamp
SIG = 1.0 / 16.0          # row scale
C_PER = 16                # consts columns per (layer, group-tile)

F16 = mybir.dt.float16
F32 = mybir.dt.float32
F8 = mybir.dt.float8e4
AF = mybir.ActivationFunctionType
FP8 = ml_dtypes.float8_e4m3

# consts col idx within a (layer, gt) block
(SB1, YB0, SB0, C10, C11, C12, C20, C21, C22, LOL, HIL, MUH, E1P, E2P,
 NLAM) = range(15)

LAST_RESULTS = None  # BassKernelResults of the most recent run (for test.py)

_Bm = np.array([[0.0, 1, 0], [0, -1, 1], [-1, 1, 0]])
_T = np.array([[1.0, 0, 0], [0, 0.5, 0.5], [0, 0.5, -0.5]])


def _q8(x):
    return np.clip(x, -240, 240).astype(FP8)


def _prepare(inputs):
    inp = {k: np.asarray(v) for k, v in inputs.items()}
    scale = float(np.abs(inp["scale"]))  # SCALE_FACTOR = 1.0
    layer_scale = scale ** (1.0 / N_LAYERS)
    host = {}
    consts = np.zeros((128, N_LAYERS * GT * C_PER), np.float64)
    rt, s_list = [], []
    prev_C = None
    prev_perm = None
    for i in range(N_LAYERS):
        wpn = inp[f"w{i}"].astype(np.float64)
        raw_w = (wpn[:D_IN] - wpn[D_IN:]) * layer_scale          # [1024, 2048]
        w_re = np.concatenate([raw_w[:, 0::2], raw_w[:, 1::2]], axis=1)
        bias = inp[f"b{i}"].astype(np.float64)
        b_re = np.concatenate([bias[0::2], bias[1::2]])
        if prev_C is not None:
            b_re = b_re + prev_C @ w_re
            w_re = w_re[prev_perm]       # rows follow prev layer's output order
        b0f, b1f = b_re[:D_IN], b_re[D_IN:]
        a = inp[f"a{i}"].astype(np.float64)[:, :, 0]             # [1024, 9]
        M = a.reshape(D_IN, 3, 3).transpose(0, 2, 1)             # [g, c0, c1]
        K = np.einsum("ri,gij,cj->grc", _Bm, M, _Bm)             # {1,u,v}
        Kt = np.einsum("sr,grc,ct->gst", _T, K, _T)              # {1,t,a}
        A11, A21 = Kt[:, 1, 1], Kt[:, 2, 1]
        A12, A22 = Kt[:, 1, 2], Kt[:, 2, 2]
        det = A11 * A22 - A21 * A12
        safe = np.abs(det) > 1e-12
        dets = np.where(safe, det, 1.0)
        lam = np.where(safe, (Kt[:, 0, 1] * A22 - Kt[:, 0, 2] * A21) / dets, 0.0)
        mu = np.where(safe, (A11 * Kt[:, 0, 2] - A12 * Kt[:, 0, 1]) / dets, 0.0)
        lam = np.clip(lam, -CL, CL)
        mu = np.clip(mu, -CL, CL)
        e1 = Kt[:, 0, 1] - (lam * A11 + mu * A21)
        e2 = Kt[:, 0, 2] - (lam * A12 + mu * A22)
        e1[np.abs(e1) < 1e-7] = 0.0
        e2[np.abs(e2) < 1e-7] = 0.0
        resid = (np.abs(e1) > 0) | (np.abs(e2) > 0)
        p_sorted = np.argsort(resid, kind="stable")   # clean first, resid last
        n_res = int(resid.sum())
        n_rt = int(np.ceil(n_res / 128))
        # place the residual tile(s) mid-layer (tile RT_POS..) so they sit
        # off the layer-boundary critical path
        rpos = min(3, GT - n_rt)
        clean = p_sorted[:D_IN - n_res]
        resg = p_sorted[D_IN - n_res:]
        perm = np.concatenate([clean[:rpos * 128], resg,
                               clean[rpos * 128 + (n_rt * 128 - n_res) or len(clean):]
                               if False else
                               clean[rpos * 128 - 0:]])
        # pad: resid block occupies n_rt*128 slots; fill remainder with clean
        perm = np.concatenate([clean[:rpos * 128],
                               resg,
                               clean[rpos * 128:rpos * 128 + n_rt * 128 - n_res],
                               clean[rpos * 128 + n_rt * 128 - n_res:]])
        rt.append((rpos, n_rt))
        C = Kt[:, 0, 0] - lam * Kt[:, 1, 0] - mu * Kt[:, 2, 0]  # old order

        wp = w_re[:, np.concatenate([perm, D_IN + perm])]
        b0, b1 = b0f[perm], b1f[perm]
        Ktp, lamp, mup = Kt[perm], lam[perm], mu[perm]
        e1p_, e2p_ = e1[perm], e2[perm]

        s = float(2 ** min(int(np.floor(np.log2(192.0 / np.abs(wp).max()))), 12))
        s_list.append(s)
        host[f"w{i}"] = np.ascontiguousarray(
            _q8(wp * s).reshape(KT, 128, FT, 128).transpose(1, 2, 0, 3))
        cols = np.stack([
            s * b1, s * (b0 + lamp), s * b0,                   # SB1 YB0 SB0
            SIG * Ktp[:, 1, 0], SIG * Ktp[:, 1, 1] / s, SIG * Ktp[:, 1, 2] / s,
            SIG * Ktp[:, 2, 0], SIG * Ktp[:, 2, 1] / s, SIG * Ktp[:, 2, 2] / s,
            s * (lamp - 1), s * (lamp + 1),                    # LOL HIL
            s * mup,                                           # MUH
            SIG * e1p_, SIG * e2p_,                            # E1P E2P
            -s * lamp, np.zeros(D_IN),                         # NLAM
        ], axis=1)                                             # [1024, 16]
        consts[:, i * GT * C_PER:(i + 1) * GT * C_PER] = (
            cols.reshape(GT, 128, C_PER).transpose(1, 0, 2).reshape(128, -1))
        prev_C = C
        prev_perm = perm
    host["consts"] = np.ascontiguousarray(consts.astype(np.float32))
    xq = _q8(inp["X"].astype(np.float32))
    xt = np.ascontiguousarray(
        xq.T.reshape(KT, 128, N_TOTAL).transpose(1, 0, 2))
    addvec = prev_C + inp["out_bias"].astype(np.float64)       # old order
    return host, xt, tuple(rt), tuple(s_list), addvec, prev_perm


def _build(rt, s_list):
    nc = bacc.Bacc("TRN2", debug=False)
    x0_d = nc.dram_tensor("x0", [128, KT, NS], F8, kind="ExternalInput")
    w_d = [nc.dram_tensor(f"w{i}", [128, FT, KT, 128], F8, kind="ExternalInput")
           for i in range(N_LAYERS)]
    c_d = nc.dram_tensor("consts", [128, N_LAYERS * GT * C_PER], F32,
                         kind="ExternalInput")
    out_d = nc.dram_tensor("outT", [GT, 128, NS], F16, kind="ExternalOutput")

    with tile.TileContext(nc) as tc:
        with (
            tc.tile_pool(name="w", bufs=1) as wpool,
            tc.tile_pool(name="x", bufs=1) as xpool,
            tc.tile_pool(name="c", bufs=1) as cpool,
            tc.tile_pool(name="ew", bufs=1) as ew,
            tc.tile_pool(name="ps", bufs=1, space="PSUM") as pspool,
        ):
            ct = cpool.tile([128, N_LAYERS * GT * C_PER], F32, tag="c")
            nc.sync.dma_start(ct[:], c_d[:])

            warm = cpool.tile([128, 1], F32, tag="warm")
            nc.vector.memset(warm[:], 0.0)
            nc.scalar.activation(warm[:], warm[:], AF.Identity, bias=0.0,
                                 scale=1.0)

            def col(layer, gt, c):
                base = layer * GT * C_PER + gt * C_PER + c
                return ct[:, base:base + 1]

            def load_w(layer, split=False):
                t = wpool.tile([128, FT * KT * 128], F8, tag=f"w{layer % 2}",
                               name=f"wt{layer}")
                tv = t[:].rearrange("p (f k c) -> p f k c", f=FT, k=KT)
                if split:
                    for gt in range(GT):
                        for ft in (GT + gt, gt):   # x1 tile first (first use)
                            nc.sync.dma_start(tv[:, ft], w_d[layer][:, ft])
                else:
                    nc.sync.dma_start(tv, w_d[layer][:])
                return t

            # x split across both HWDGE rings so the layer-0 input lands
            # ~2x sooner; tile-0's weight f-tiles go first on the sync ring.
            xa = xpool.tile([128, KT * NS], F8, tag="xA", name="xin0")
            xav = xa[:].rearrange("p (k n) -> p k n", k=KT)
            w_cur = wpool.tile([128, FT * KT * 128], F8, tag="w0", name="wt0")
            wv0 = w_cur[:].rearrange("p (f k c) -> p f k c", f=FT, k=KT)
            nc.sync.dma_start(wv0[:, GT], w_d[0][:, GT])      # ft8 (x1, gt0)
            nc.sync.dma_start(wv0[:, 0], w_d[0][:, 0])        # ft0 (x0, gt0)
            for kt in range(KT):
                ring = nc.scalar if kt % 2 == 0 else nc.sync
                ring.dma_start(xav[:, kt], x0_d[:, kt])
            for gt in range(GT):
                for ft in (GT + gt, gt):
                    if ft in (GT, 0):
                        continue
                    nc.sync.dma_start(wv0[:, ft], w_d[0][:, ft])
            w_next = None

            x_tiles = {0: xa}
            state = {}

            def stage_m(layer, gt, xv, wv):
                pss = {}
                for half in (1, 0):
                    ps = pspool.tile([128, NS], F32, tag=f"ps{half}",
                                     name=f"ps{half}_{layer}_{gt}")
                    ft = half * GT + gt
                    for kp in range(KT // 2):
                        lhsT = wv[:, ft, 2 * kp:2 * kp + 2, :]
                        for pc in range(NS // PS):
                            nc.tensor.matmul(
                                ps[:, pc * PS:(pc + 1) * PS],
                                lhsT,
                                xv[:, 2 * kp:2 * kp + 2, pc * PS:(pc + 1) * PS],
                                start=(kp == 0), stop=(kp == KT // 2 - 1),
                                perf_mode=mybir.MatmulPerfMode.DoubleRow,
                            )
                    pss[half] = ps
                state[(layer, gt)] = {"ps": pss}

            def stage_evac(layer, gt):
                """ACT: psum evacs y1 = ps1+s*b1, y0' = ps0+s*(b0+lam),
                ab0 = |ps0+s*b0| (all fp16)."""
                st = state[(layer, gt)]
                y1 = ew.tile([128, NS], F16, tag="y1", bufs=2)
                nc.scalar.activation(y1[:], st["ps"][1][:], AF.Identity,
                                     bias=col(layer, gt, SB1), scale=1.0)
                y0 = ew.tile([128, NS], F16, tag="y0", bufs=2)
                nc.scalar.activation(y0[:], st["ps"][0][:], AF.Identity,
                                     bias=col(layer, gt, YB0), scale=1.0)
                # |ps0 + s*b0| = |y0 - s*lam|: read the SBUF copy so ps0's
                # PSUM banks free after a single pass (shortens the WAR chain
                # gating the next tile's x0 matmuls)
                ab0 = ew.tile([128, NS], F16, tag="ab0", bufs=2)
                nc.scalar.activation(ab0[:], y0[:], AF.Abs,
                                     bias=col(layer, gt, NLAM), scale=1.0)
                st.update(y1=y1, y0=y0, ab0=ab0)

            def stage_clip(layer, gt):
                """DVE: t1 = clip(y1,+-s); L0 = clip(y0', lam_h +- s);
                A0 = min(ab0, s) + mu_h."""
                st = state[(layer, gt)]
                s = s_list[layer]
                t1 = ew.tile([128, NS], F16, tag="t1", bufs=3)
                nc.vector.tensor_scalar(t1[:], st["y1"][:], -s, s,
                                        AluOpType.max, AluOpType.min)
                L0 = ew.tile([128, NS], F16, tag="L0", bufs=3)
                nc.vector.tensor_scalar(L0[:], st["y0"][:],
                                        col(layer, gt, LOL),
                                        col(layer, gt, HIL),
                                        AluOpType.max, AluOpType.min)
                A0 = ew.tile([128, NS], F16, tag="A0", bufs=3)
                nc.vector.tensor_scalar(A0[:], st["ab0"][:], s,
                                        col(layer, gt, MUH),
                                        AluOpType.min, AluOpType.add)
                st.update(t1=t1, L0=L0, A0=A0)

            def stage_row(layer, gt):
                """ACT: a1 = |t1|.  DVE: row t-affines g1, g2."""
                st = state[(layer, gt)]
                a1 = ew.tile([128, NS], F16, tag="a1", bufs=2)
                nc.scalar.activation(a1[:], st["t1"][:], AF.Abs, bias=0.0,
                                     scale=1.0)
                g1 = ew.tile([128, NS], F16, tag="g1", bufs=2)
                if layer == N_LAYERS - 1:
                    # no fp8 convert on the last layer: ACT has slack, DVE is
                    # the bottleneck -> do one row affine on ACT
                    nc.scalar.activation(g1[:], st["t1"][:], AF.Identity,
                                         bias=col(layer, gt, C10),
                                         scale=col(layer, gt, C11))
                else:
                    nc.vector.tensor_scalar(g1[:], st["t1"][:],
                                            col(layer, gt, C11),
                                            col(layer, gt, C10),
                                            AluOpType.mult, AluOpType.add)
                g2 = ew.tile([128, NS], F16, tag="g2", bufs=2)
                nc.vector.tensor_scalar(g2[:], st["t1"][:],
                                        col(layer, gt, C21),
                                        col(layer, gt, C20),
                                        AluOpType.mult, AluOpType.add)
                st.update(a1=a1, g1=g1, g2=g2)

            def stage_comb(layer, gt):
                """DVE: abs-parts, rows, products, final add (in-place)."""
                st = state[(layer, gt)]
                g1, g2, a1 = st["g1"], st["g2"], st["a1"]
                z1 = ew.tile([128, NS], F16, tag="z1", bufs=2)
                nc.vector.tensor_scalar(z1[:], a1[:], col(layer, gt, C12),
                                        None, AluOpType.mult)
                z2 = ew.tile([128, NS], F16, tag="z2", bufs=2)
                nc.vector.tensor_scalar(z2[:], a1[:], col(layer, gt, C22),
                                        None, AluOpType.mult)
                nc.vector.tensor_tensor(g1[:], z1[:], g1[:],
                                        AluOpType.add)     # R1
                nc.vector.tensor_tensor(g2[:], z2[:], g2[:],
                                        AluOpType.add)     # R2
                nc.vector.tensor_tensor(g1[:], st["L0"][:], g1[:],
                                        AluOpType.mult)    # p1
                nc.vector.tensor_tensor(g2[:], st["A0"][:], g2[:],
                                        AluOpType.mult)    # p2
                if gt % 2 == 0:
                    fpair = ew.tile([128, 2 * NS], F16, tag="fp", bufs=2)
                    st["fpair"] = fpair
                else:
                    fpair = state[(layer, gt - 1)]["fpair"]
                    st["fpair"] = fpair
                fh = fpair[:, (gt % 2) * NS:(gt % 2 + 1) * NS]
                if rt[layer][0] <= gt < rt[layer][0] + rt[layer][1]:
                    z3 = ew.tile([128, NS], F16, tag="z1", bufs=2, name="z3")
                    nc.vector.tensor_scalar(z3[:], st["t1"][:],
                                            col(layer, gt, E1P), None,
                                            AluOpType.mult)
                    z4 = ew.tile([128, NS], F16, tag="z2", bufs=2, name="z4")
                    nc.vector.tensor_scalar(z4[:], a1[:],
                                            col(layer, gt, E2P), None,
                                            AluOpType.mult)
                    nc.vector.tensor_tensor(fh, g1[:], g2[:], AluOpType.add)
                    nc.vector.tensor_tensor(fh, z3[:], fh, AluOpType.add)
                    nc.vector.tensor_tensor(fh, z4[:], fh, AluOpType.add)
                else:
                    nc.vector.tensor_tensor(fh, g1[:], g2[:], AluOpType.add)

            def stage_out(layer, gt):
                """Odd gt: fp8 convert of the pair (layers 0,1);
                last layer: per-gt output DMA."""
                if layer == N_LAYERS - 1:
                    st = state.pop((layer, gt))
                    fh = st["fpair"][:, (gt % 2) * NS:(gt % 2 + 1) * NS]
                    nc.sync.dma_start(out_d[gt], fh)
                    return
                st = state.pop((layer, gt))
                if gt % 2 == 0:
                    return
                xn = st["xnext"]
                sc = 1.0 / (s_list[layer] * SIG)
                if gt == GT - 1:
                    # last pair split into two converts so the first fires
                    # a tile earlier (shortens the layer-boundary chain)
                    fpair = st["fpair"]
                    nc.scalar.activation(xn[:, (gt - 1) * NS:gt * NS],
                                         fpair[:, :NS], AF.Identity,
                                         bias=0.0, scale=sc)
                    nc.scalar.activation(xn[:, gt * NS:(gt + 1) * NS],
                                         fpair[:, NS:], AF.Identity,
                                         bias=0.0, scale=sc)
                else:
                    fpair = st["fpair"]
                    nc.scalar.activation(
                        xn[:, (gt - 1) * NS:(gt + 1) * NS], fpair[:],
                        AF.Identity, bias=0.0, scale=sc)

            for layer in range(N_LAYERS):
                xin = x_tiles[layer % 2]
                xv = xin[:].rearrange("p (k n) -> p k n", k=KT)
                wv = w_cur[:].rearrange("p (f k c) -> p f k c", f=FT, k=KT)
                if layer < N_LAYERS - 1:
                    xnext = xpool.tile([128, KT * NS], F8,
                                       tag=f"x{'A' if layer % 2 == 1 else 'B'}",
                                       name=f"xin{layer + 1}")
                    x_tiles[(layer + 1) % 2] = xnext
                stages = [stage_evac, stage_clip, stage_row, stage_comb,
                          stage_out]
                # per-step emission order: ACT evac first (frees PSUM for the
                # next M), then DVE work oldest-tile-first so the DVE FIFO
                # never head-blocks on a just-produced psum, cvt last.
                order = [(0, stage_evac), (3, stage_comb), (2, stage_row),
                         (1, stage_clip), (4, stage_out)]
                for gt in range(GT):
                    for d, fn in order:
                        if gt >= d + 1:
                            fn(layer, gt - d - 1)
                    stage_m(layer, gt, xv, wv)
                    if layer < N_LAYERS - 1:
                        state[(layer, gt)]["xnext"] = xnext
                    if gt == 0 and layer + 1 < N_LAYERS:
                        w_next = load_w(layer + 1)
                # drain the pipeline (keep per-step stage order causal)
                for step in range(GT, GT + len(stages)):
                    for d, fn in order:
                        g = step - d - 1
                        if GT - d - 1 <= g < GT:
                            fn(layer, g)
                if layer + 1 < N_LAYERS:
                    if w_next is None:
                        pass
                    w_cur = w_next
    nc.compile()
    return nc


_NC_CACHE = {}


def _get_nc(rt, s_list):
    key = (rt, s_list)
    if key not in _NC_CACHE:
        _NC_CACHE[key] = _build(rt, s_list)
    return _NC_CACHE[key]


def kernel(**inputs):
    global LAST_RESULTS
    host, xt, rt, s_list, addvec, last_perm = _prepare(inputs)
    nc = _get_nc(rt, s_list)
    in_maps = []
    for core in range(N_CORES):
        m = dict(host)
        m["x0"] = np.ascontiguousarray(xt[:, :, core * NS:(core + 1) * NS])
        in_maps.append(m)
    want_trace = bool(os.environ.get("BASS_TRACE"))
    os.environ["BASS_NEVER_TRACE"] = "1"
    try:
        res = run_bass_kernel_spmd(
            nc, in_maps, core_ids=list(range(N_CORES)), trace=False
        )
    finally:
        del os.environ["BASS_NEVER_TRACE"]
    LAST_RESULTS = res
    if want_trace:
        try:
            LAST_RESULTS = run_bass_kernel_spmd(
                nc, in_maps, core_ids=list(range(N_CORES)), trace=True
            )
        except Exception as e:  # profiling is best-effort
            print("trace run failed:", e)
    xnew = np.concatenate(
        [r["outT"].reshape(D_IN, NS) for r in res.results], axis=1)
    descale = 1.0 / (s_list[-1] * SIG)
    out = np.empty((N_TOTAL, D_IN), np.float32)
    out[:, last_perm] = (xnew.T.astype(np.float64) * descale
                         + addvec[last_perm][None, :]).astype(np.float32)
    return out


# revision 13
# speedup vs baseline: 1.0028x; 1.0028x over previous
"""Trainium2 Bass kernel for nn_Network_85220741087986 (v3: fp8 DoubleRow).

3-layer MLP: per layer  X[N,1024] @ W[1024,2048] -> per-group bilinear
interpolation on a 3x3 grid (ARITY=2) -> X[N,1024].

Reformulation (host-validated, rel err ~1.8e-3 vs fp32 reference):
  - Matmuls in fp8 e4m3 with perf_mode=DoubleRow (2 k-tiles per
    instruction); weights scaled by s=2^12 per layer, PSUM holds s*preact.
  - Activation in the {1, t, a} basis, t = clip(y,-1,1), a = |t|:
      f = (t0+lam)*ROW1 + (a0+mu)*ROW2 + C
      ROW_r = K_r0 + K_r1 t1 + K_r2 a1
    lam/mu solve a per-group 2x2 system (clamped to |.|<=8); clamped
    groups get a residual  + e1*t1 + e2*a1  and are permuted into the
    last group-tile of each layer so only that tile pays for it.  C and
    the per-feature bias fold into the next layer's bias shift; rows are
    scaled by sigma=1/16 so all fp16 intermediates stay in range, and the
    1/(s*sigma) descale rides the fp8-convert's free ACT scale slot.
  - Engine split per group-tile [128 groups x 2048 samples], chosen from
    measured op costs (DVE TS 4x ~660ns, TT 2x ~1150ns, ACT ~2000ns,
    scalar_tensor_tensor and PSUM-source DVE ops are 1x -> avoided):
      PE : 32 DoubleRow matmuls
      ACT: y1/y0 = psum + s*b (PSUM evac, fp16), h1/h2 row affines,
           fp8 convert (paired across 2 group-tiles, FD=4096)
      DVE: 2 symmetric clips (float bounds), scaled-abs ops (abs_max
           trick), 5 tensor_tensor ops, all 4x/2x SBUF fp16
  - 5-stage software pipeline across group-tiles (M, evac, clip/abs,
    row-affine, combine, convert) so no engine head-blocks on a
    same-tile cross-engine product.

Sharding: pure data parallel over 8 cores (2048 samples each), weights
and constants replicated.
"""

import os
import sys

import numpy as np
import ml_dtypes

for _p in ("/opt/trn_rl_repo", "/root/.axon_site/_ro/trn_rl_repo"):
    if os.path.isdir(_p) and _p not in sys.path:
        sys.path.append(_p)

import concourse.bacc as bacc
import concourse.mybir as mybir
import concourse.tile as tile
from concourse.alu_op_type import AluOpType
from concourse.bass_utils import run_bass_kernel_spmd


def _ensure_axon_hooks():
    """This image's antenv lacks axon_hooks; provide it (and register the
    NTFF profile hook) so trace=True doesn't crash run_bass_kernel_spmd."""
    import types

    try:
        import antenv.axon_hooks  # noqa: F401
        return
    except ImportError:
        pass
    mod = types.ModuleType("antenv.axon_hooks")
    _hook = [None]
    mod.get_axon_ntff_profile_hook = lambda: _hook[0]
    mod.set_axon_ntff_profile_hook = lambda h: _hook.__setitem__(0, h)
    sys.modules["antenv.axon_hooks"] = mod
    try:
        import antenv
        antenv.axon_hooks = mod
    except ImportError:
        pass
    try:
        from trn_agent_boot.trn_boot import _ntff_profile_via_ctypes
        so = "/opt/axon/libaxon_pjrt.so"
        if os.path.exists(so):
            _hook[0] = _ntff_profile_via_ctypes(so)
    except Exception:
        pass


_ensure_axon_hooks()

N_TOTAL = 16384
D_IN = 1024
F_OUT = 2048
N_LAYERS = 3
N_CORES = 8
NS = N_TOTAL // N_CORES   # samples per core
KT = D_IN // 128          # 8 contraction tiles
FT = F_OUT // 128         # 16 matmul-output feature tiles
GT = D_IN // 128          # 8 group tiles
PS = 512                  # psum chunk (one fp32 bank)
CL = 8.0                  # lambda/mu clamp
SIG = 1.0 / 16.0          # row scale
C_PER = 16                # consts columns per (layer, group-tile)

F16 = mybir.dt.float16
F32 = mybir.dt.float32
F8 = mybir.dt.float8e4
AF = mybir.ActivationFunctionType
FP8 = ml_dtypes.float8_e4m3

# consts col idx within a (layer, gt) block
(SB1, YB0, SB0, C10, C11, C12, C20, C21, C22, LOL, HIL, MUH, E1P, E2P,
 NLAM) = range(15)

LAST_RESULTS = None  # BassKernelResults of the most recent run (for test.py)

_Bm = np.array([[0.0, 1, 0], [0, -1, 1], [-1, 1, 0]])
_T = np.array([[1.0, 0, 0], [0, 0.5, 0.5], [0, 0.5, -0.5]])


def _q8(x):
    return np.clip(x, -240, 240).astype(FP8)


def _prepare(inputs):
    inp = {k: np.asarray(v) for k, v in inputs.items()}
    scale = float(np.abs(inp["scale"]))  # SCALE_FACTOR = 1.0
    layer_scale = scale ** (1.0 / N_LAYERS)
    host = {}
    consts = np.zeros((128, N_LAYERS * GT * C_PER), np.float64)
    rt, s_list = [], []
    prev_C = None
    prev_perm = None
    for i in range(N_LAYERS):
        wpn = inp[f"w{i}"].astype(np.float64)
        raw_w = (wpn[:D_IN] - wpn[D_IN:]) * layer_scale          # [1024, 2048]
        w_re = np.concatenate([raw_w[:, 0::2], raw_w[:, 1::2]], axis=1)
        bias = inp[f"b{i}"].astype(np.float64)
        b_re = np.concatenate([bias[0::2], bias[1::2]])
        if prev_C is not None:
            b_re = b_re + prev_C @ w_re
            w_re = w_re[prev_perm]       # rows follow prev layer's output order
        b0f, b1f = b_re[:D_IN], b_re[D_IN:]
        a = inp[f"a{i}"].astype(np.float64)[:, :, 0]             # [1024, 9]
        M = a.reshape(D_IN, 3, 3).transpose(0, 2, 1)             # [g, c0, c1]
        K = np.einsum("ri,gij,cj->grc", _Bm, M, _Bm)             # {1,u,v}
        Kt = np.einsum("sr,grc,ct->gst", _T, K, _T)              # {1,t,a}
        A11, A21 = Kt[:, 1, 1], Kt[:, 2, 1]
        A12, A22 = Kt[:, 1, 2], Kt[:, 2, 2]
        det = A11 * A22 - A21 * A12
        safe = np.abs(det) > 1e-12
        dets = np.where(safe, det, 1.0)
        lam = np.where(safe, (Kt[:, 0, 1] * A22 - Kt[:, 0, 2] * A21) / dets, 0.0)
        mu = np.where(safe, (A11 * Kt[:, 0, 2] - A12 * Kt[:, 0, 1]) / dets, 0.0)
        lam = np.clip(lam, -CL, CL)
        mu = np.clip(mu, -CL, CL)
        e1 = Kt[:, 0, 1] - (lam * A11 + mu * A21)
        e2 = Kt[:, 0, 2] - (lam * A12 + mu * A22)
        e1[np.abs(e1) < 1e-7] = 0.0
        e2[np.abs(e2) < 1e-7] = 0.0
        resid = (np.abs(e1) > 0) | (np.abs(e2) > 0)
        p_sorted = np.argsort(resid, kind="stable")   # clean first, resid last
        n_res = int(resid.sum())
        n_rt = int(np.ceil(n_res / 128))
        # place the residual tile(s) mid-layer (tile RT_POS..) so they sit
        # off the layer-boundary critical path
        rpos = min(3, GT - n_rt)
        clean = p_sorted[:D_IN - n_res]
        resg = p_sorted[D_IN - n_res:]
        perm = np.concatenate([clean[:rpos * 128], resg,
                               clean[rpos * 128 + (n_rt * 128 - n_res) or len(clean):]
                               if False else
                               clean[rpos * 128 - 0:]])
        # pad: resid block occupies n_rt*128 slots; fill remainder with clean
        perm = np.concatenate([clean[:rpos * 128],
                               resg,
                               clean[rpos * 128:rpos * 128 + n_rt * 128 - n_res],
                               clean[rpos * 128 + n_rt * 128 - n_res:]])
        rt.append((rpos, n_rt))
        C = Kt[:, 0, 0] - lam * Kt[:, 1, 0] - mu * Kt[:, 2, 0]  # old order

        wp = w_re[:, np.concatenate([perm, D_IN + perm])]
        b0, b1 = b0f[perm], b1f[perm]
        Ktp, lamp, mup = Kt[perm], lam[perm], mu[perm]
        e1p_, e2p_ = e1[perm], e2[perm]

        s = float(2 ** min(int(np.floor(np.log2(192.0 / np.abs(wp).max()))), 12))
        s_list.append(s)
        host[f"w{i}"] = np.ascontiguousarray(
            _q8(wp * s).reshape(KT, 128, FT, 128).transpose(1, 2, 0, 3))
        cols = np.stack([
            s * b1, s * (b0 + lamp), s * b0,                   # SB1 YB0 SB0
            SIG * Ktp[:, 1, 0], SIG * Ktp[:, 1, 1] / s, SIG * Ktp[:, 1, 2] / s,
            SIG * Ktp[:, 2, 0], SIG * Ktp[:, 2, 1] / s, SIG * Ktp[:, 2, 2] / s,
            s * (lamp - 1), s * (lamp + 1),                    # LOL HIL
            s * mup,                                           # MUH
            SIG * e1p_, SIG * e2p_,                            # E1P E2P
            -s * lamp, np.zeros(D_IN),                         # NLAM
        ], axis=1)                                             # [1024, 16]
        consts[:, i * GT * C_PER:(i + 1) * GT * C_PER] = (
            cols.reshape(GT, 128, C_PER).transpose(1, 0, 2).reshape(128, -1))
        prev_C = C
        prev_perm = perm
    host["consts"] = np.ascontiguousarray(consts.astype(np.float32))
    xq = _q8(inp["X"].astype(np.float32))
    xt = np.ascontiguousarray(
        xq.T.reshape(KT, 128, N_TOTAL).transpose(1, 0, 2))
    addvec = prev_C + inp["out_bias"].astype(np.float64)       # old order
    return host, xt, tuple(rt), tuple(s_list), addvec, prev_perm


def _build(rt, s_list):
    nc = bacc.Bacc("TRN2", debug=False)
    x0_d = nc.dram_tensor("x0", [128, KT, NS], F8, kind="ExternalInput")
    w_d = [nc.dram_tensor(f"w{i}", [128, FT, KT, 128], F8, kind="ExternalInput")
           for i in range(N_LAYERS)]
    c_d = nc.dram_tensor("consts", [128, N_LAYERS * GT * C_PER], F32,
                         kind="ExternalInput")
    out_d = nc.dram_tensor("outT", [GT, 128, NS], F16, kind="ExternalOutput")

    with tile.TileContext(nc) as tc:
        with (
            tc.tile_pool(name="w", bufs=1) as wpool,
            tc.tile_pool(name="x", bufs=1) as xpool,
            tc.tile_pool(name="c", bufs=1) as cpool,
            tc.tile_pool(name="ew", bufs=1) as ew,
            tc.tile_pool(name="ps", bufs=1, space="PSUM") as pspool,
        ):
            ct = cpool.tile([128, N_LAYERS * GT * C_PER], F32, tag="c")
            nc.sync.dma_start(ct[:], c_d[:])

            warm = cpool.tile([128, 1], F32, tag="warm")
            nc.vector.memset(warm[:], 0.0)
            nc.scalar.activation(warm[:], warm[:], AF.Identity, bias=0.0,
                                 scale=1.0)

            def col(layer, gt, c):
                base = layer * GT * C_PER + gt * C_PER + c
                return ct[:, base:base + 1]

            def load_w(layer, split=False):
                t = wpool.tile([128, FT * KT * 128], F8, tag=f"w{layer % 2}",
                               name=f"wt{layer}")
                tv = t[:].rearrange("p (f k c) -> p f k c", f=FT, k=KT)
                if split:
                    for gt in range(GT):
                        for ft in (GT + gt, gt):   # x1 tile first (first use)
                            nc.sync.dma_start(tv[:, ft], w_d[layer][:, ft])
                else:
                    nc.sync.dma_start(tv, w_d[layer][:])
                return t

            # x split across both HWDGE rings so the layer-0 input lands
            # ~2x sooner; tile-0's weight f-tiles go first on the sync ring.
            xa = xpool.tile([128, KT * NS], F8, tag="xA", name="xin0")
            xav = xa[:].rearrange("p (k n) -> p k n", k=KT)
            w_cur = wpool.tile([128, FT * KT * 128], F8, tag="w0", name="wt0")
            wv0 = w_cur[:].rearrange("p (f k c) -> p f k c", f=FT, k=KT)
            nc.sync.dma_start(wv0[:, GT], w_d[0][:, GT])      # ft8 (x1, gt0)
            nc.sync.dma_start(wv0[:, 0], w_d[0][:, 0])        # ft0 (x0, gt0)
            for kt in range(KT):
                ring = nc.scalar if kt % 2 == 0 else nc.sync
                ring.dma_start(xav[:, kt], x0_d[:, kt])
            for gt in range(GT):
                for ft in (GT + gt, gt):
                    if ft in (GT, 0):
                        continue
                    nc.sync.dma_start(wv0[:, ft], w_d[0][:, ft])
            w_next = None

            x_tiles = {0: xa}
            state = {}

            def stage_m(layer, gt, xv, wv):
                pss = {}
                for half in (1, 0):
                    ps = pspool.tile([128, NS], F32, tag=f"ps{half}",
                                     name=f"ps{half}_{layer}_{gt}")
                    ft = half * GT + gt
                    for kp in range(KT // 2):
                        lhsT = wv[:, ft, 2 * kp:2 * kp + 2, :]
                        for pc in range(NS // PS):
                            nc.tensor.matmul(
                                ps[:, pc * PS:(pc + 1) * PS],
                                lhsT,
                                xv[:, 2 * kp:2 * kp + 2, pc * PS:(pc + 1) * PS],
                                start=(kp == 0), stop=(kp == KT // 2 - 1),
                                perf_mode=mybir.MatmulPerfMode.DoubleRow,
                            )
                    pss[half] = ps
                state[(layer, gt)] = {"ps": pss}

            def stage_evac(layer, gt):
                """ACT: psum evacs y1 = ps1+s*b1, y0' = ps0+s*(b0+lam),
                ab0 = |ps0+s*b0| (all fp16)."""
                st = state[(layer, gt)]
                y1 = ew.tile([128, NS], F16, tag="y1", bufs=2)
                nc.scalar.activation(y1[:], st["ps"][1][:], AF.Identity,
                                     bias=col(layer, gt, SB1), scale=1.0)
                y0 = ew.tile([128, NS], F16, tag="y0", bufs=2)
                nc.scalar.activation(y0[:], st["ps"][0][:], AF.Identity,
                                     bias=col(layer, gt, YB0), scale=1.0)
                # |ps0 + s*b0| = |y0 - s*lam|: read the SBUF copy so ps0's
                # PSUM banks free after a single pass (shortens the WAR chain
                # gating the next tile's x0 matmuls)
                ab0 = ew.tile([128, NS], F16, tag="ab0", bufs=2)
                nc.scalar.activation(ab0[:], y0[:], AF.Abs,
                                     bias=col(layer, gt, NLAM), scale=1.0)
                st.update(y1=y1, y0=y0, ab0=ab0)

            def stage_clip(layer, gt):
                """DVE: t1 = clip(y1,+-s); L0 = clip(y0', lam_h +- s);
                A0 = min(ab0, s) + mu_h."""
                st = state[(layer, gt)]
                s = s_list[layer]
                t1 = ew.tile([128, NS], F16, tag="t1", bufs=3)
                nc.vector.tensor_scalar(t1[:], st["y1"][:], -s, s,
                                        AluOpType.max, AluOpType.min)
                L0 = ew.tile([128, NS], F16, tag="L0", bufs=3)
                nc.vector.tensor_scalar(L0[:], st["y0"][:],
                                        col(layer, gt, LOL),
                                        col(layer, gt, HIL),
                                        AluOpType.max, AluOpType.min)
                A0 = ew.tile([128, NS], F16, tag="A0", bufs=3)
                nc.vector.tensor_scalar(A0[:], st["ab0"][:], s,
                                        col(layer, gt, MUH),
                                        AluOpType.min, AluOpType.add)
                st.update(t1=t1, L0=L0, A0=A0)

            def stage_row(layer, gt):
                """ACT: a1 = |t1|.  DVE: row t-affines g1, g2."""
                st = state[(layer, gt)]
                a1 = ew.tile([128, NS], F16, tag="a1", bufs=2)
                nc.scalar.activation(a1[:], st["t1"][:], AF.Abs, bias=0.0,
                                     scale=1.0)
                g1 = ew.tile([128, NS], F16, tag="g1", bufs=2)
                nc.vector.tensor_scalar(g1[:], st["t1"][:],
                                        col(layer, gt, C11),
                                        col(layer, gt, C10),
                                        AluOpType.mult, AluOpType.add)
                g2 = ew.tile([128, NS], F16, tag="g2", bufs=2)
                nc.vector.tensor_scalar(g2[:], st["t1"][:],
                                        col(layer, gt, C21),
                                        col(layer, gt, C20),
                                        AluOpType.mult, AluOpType.add)
                st.update(a1=a1, g1=g1, g2=g2)

            def stage_comb(layer, gt):
                """DVE: abs-parts, rows, products, final add (in-place)."""
                st = state[(layer, gt)]
                g1, g2, a1 = st["g1"], st["g2"], st["a1"]
                z1 = ew.tile([128, NS], F16, tag="z1", bufs=2)
                nc.vector.tensor_scalar(z1[:], a1[:], col(layer, gt, C12),
                                        None, AluOpType.mult)
                z2 = ew.tile([128, NS], F16, tag="z2", bufs=2)
                nc.vector.tensor_scalar(z2[:], a1[:], col(layer, gt, C22),
                                        None, AluOpType.mult)
                nc.vector.tensor_tensor(g1[:], z1[:], g1[:],
                                        AluOpType.add)     # R1
                nc.vector.tensor_tensor(g2[:], z2[:], g2[:],
                                        AluOpType.add)     # R2
                nc.vector.tensor_tensor(g1[:], st["L0"][:], g1[:],
                                        AluOpType.mult)    # p1
                nc.vector.tensor_tensor(g2[:], st["A0"][:], g2[:],
                                        AluOpType.mult)    # p2
                if gt % 2 == 0:
                    fpair = ew.tile([128, 2 * NS], F16, tag="fp", bufs=2)
                    st["fpair"] = fpair
                else:
                    fpair = state[(layer, gt - 1)]["fpair"]
                    st["fpair"] = fpair
                fh = fpair[:, (gt % 2) * NS:(gt % 2 + 1) * NS]
                if rt[layer][0] <= gt < rt[layer][0] + rt[layer][1]:
                    z3 = ew.tile([128, NS], F16, tag="z1", bufs=2, name="z3")
                    nc.vector.tensor_scalar(z3[:], st["t1"][:],
                                            col(layer, gt, E1P), None,
                                            AluOpType.mult)
                    z4 = ew.tile([128, NS], F16, tag="z2", bufs=2, name="z4")
                    nc.vector.tensor_scalar(z4[:], a1[:],
                                            col(layer, gt, E2P), None,
                                            AluOpType.mult)
                    nc.vector.tensor_tensor(fh, g1[:], g2[:], AluOpType.add)
                    nc.vector.tensor_tensor(fh, z3[:], fh, AluOpType.add)
                    nc.vector.tensor_tensor(fh, z4[:], fh, AluOpType.add)
                else:
                    nc.vector.tensor_tensor(fh, g1[:], g2[:], AluOpType.add)

            def stage_out(layer, gt):
                """Odd gt: fp8 convert of the pair (layers 0,1);
                last layer: per-gt output DMA."""
                if layer == N_LAYERS - 1:
                    st = state.pop((layer, gt))
                    fh = st["fpair"][:, (gt % 2) * NS:(gt % 2 + 1) * NS]
                    nc.sync.dma_start(out_d[gt], fh)
                    return
                st = state.pop((layer, gt))
                if gt % 2 == 0:
                    return
                xn = st["xnext"]
                sc = 1.0 / (s_list[layer] * SIG)
                if gt == GT - 1:
                    # last pair split into two converts so the first fires
                    # a tile earlier (shortens the layer-boundary chain)
                    fpair = st["fpair"]
                    nc.scalar.activation(xn[:, (gt - 1) * NS:gt * NS],
                                         fpair[:, :NS], AF.Identity,
                                         bias=0.0, scale=sc)
                    nc.scalar.activation(xn[:, gt * NS:(gt + 1) * NS],
                                         fpair[:, NS:], AF.Identity,
                                         bias=0.0, scale=sc)
                else:
                    fpair = st["fpair"]
                    nc.scalar.activation(
                        xn[:, (gt - 1) * NS:(gt + 1) * NS], fpair[:],
                        AF.Identity, bias=0.0, scale=sc)

            for layer in range(N_LAYERS):
                xin = x_tiles[layer % 2]
                xv = xin[:].rearrange("p (k n) -> p k n", k=KT)
                wv = w_cur[:].rearrange("p (f k c) -> p f k c", f=FT, k=KT)
                if layer < N_LAYERS - 1:
                    xnext = xpool.tile([128, KT * NS], F8,
                                       tag=f"x{'A' if layer % 2 == 1 else 'B'}",
                                       name=f"xin{layer + 1}")
                    x_tiles[(layer + 1) % 2] = xnext
                stages = [stage_evac, stage_clip, stage_row, stage_comb,
                          stage_out]
                # per-step emission order: ACT evac first (frees PSUM for the
                # next M), then DVE work oldest-tile-first so the DVE FIFO
                # never head-blocks on a just-produced psum, cvt last.
                order = [(0, stage_evac), (3, stage_comb), (2, stage_row),
                         (1, stage_clip), (4, stage_out)]
                for gt in range(GT):
                    for d, fn in order:
                        if gt >= d + 1:
                            fn(layer, gt - d - 1)
                    stage_m(layer, gt, xv, wv)
                    if layer < N_LAYERS - 1:
                        state[(layer, gt)]["xnext"] = xnext
                    if gt == 0 and layer + 1 < N_LAYERS:
                        w_next = load_w(layer + 1)
                # drain the pipeline (keep per-step stage order causal)
                for step in range(GT, GT + len(stages)):
                    for d, fn in order:
                        g = step - d - 1
                        if GT - d - 1 <= g < GT:
                            fn(layer, g)
                if layer + 1 < N_LAYERS:
                    if w_next is None:
                        pass
                    w_cur = w_next
    nc.compile()
    return nc


_NC_CACHE = {}


def _get_nc(rt, s_list):
    key = (rt, s_list)
    if key not in _NC_CACHE:
        _NC_CACHE[key] = _build(rt, s_list)
    return _NC_CACHE[key]


def kernel(**inputs):
    global LAST_RESULTS
    host, xt, rt, s_list, addvec, last_perm = _prepare(inputs)
    nc = _get_nc(rt, s_list)
    in_maps = []
    for core in range(N_CORES):
        m = dict(host)
        m["x0"] = np.ascontiguousarray(xt[:, :, core * NS:(core + 1) * NS])
        in_maps.append(m)
    want_trace = bool(os.environ.get("BASS_TRACE"))
    os.environ["BASS_NEVER_TRACE"] = "1"
    try:
        res = run_bass_kernel_spmd(
            nc, in_maps, core_ids=list(range(N_CORES)), trace=False
        )
    finally:
        del os.environ["BASS_NEVER_TRACE"]
    LAST_RESULTS = res
    if want_trace:
        try:
            LAST_RESULTS = run_bass_kernel_spmd(
                nc, in_maps, core_ids=list(range(N_CORES)), trace=True
            )
        except Exception as e:  # profiling is best-effort
            print("trace run failed:", e)
    xnew = np.concatenate(
        [r["outT"].reshape(D_IN, NS) for r in res.results], axis=1)
    descale = 1.0 / (s_list[-1] * SIG)
    out = np.empty((N_TOTAL, D_IN), np.float32)
    out[:, last_perm] = (xnew.T.astype(np.float64) * descale
                         + addvec[last_perm][None, :]).astype(np.float32)
    return out
